# revision 8
# baseline (speedup 1.0000x reference)
"""Conv2d(1->16,5x5,p2) + BN(inference) + ReLU + MaxPool2d(2) on 8 NeuronCores.

Strategy (per core, 16 images = data parallelism over batch):
  - BN is folded into the conv weights/bias on the host.
  - Conv is computed on the TensorEngine as a single matmul per 16-output-row
    slab: contraction K = (dx-block j in 0..4) x (input row yi in 0..19) = 100.
    The 5 dx shifts are materialized as 5 partition-blocks of the slab tile,
    loaded directly from HBM with column offset j (overlapping reads).
    The dy taps are encoded in a Toeplitz weight matrix lhsT[(j,yi), (o,yp)].
  - Two matmuls per slab produce even / odd output rows in separate PSUM
    banks, so the 2x2 maxpool becomes: vertical max = elementwise max of the
    two PSUM tiles (DVE), horizontal max = strided max in SBUF (GPSIMD),
    then ReLU+bias on the ScalarEngine, then DMA out.
"""

import numpy as np

import concourse.bass as bass
import concourse.bacc as bacc
import concourse.tile as tile
import concourse.mybir as mybir
from concourse.bass_utils import run_bass_kernel_spmd

F32 = mybir.dt.float32
N_CORES = 8
B, H, W = 128, 224, 224
PB = B // N_CORES          # images per core
PH, PW = H + 4, W + 4      # host-padded image
OC = 16
HO, WO = H // 2, W // 2    # 112, 112
YB = 16                    # conv output rows per slab
NT = H // YB               # 14 slabs per image pair
KROWS = YB + 4             # input rows per dx-block
K = 5 * KROWS              # 100 contraction partitions
BN_EPS = 1e-5

_CACHE: dict = {}


def _build_nc():
    nc = bacc.Bacc("TRN2", num_devices=N_CORES)
    xpad = nc.dram_tensor("xpad", [PB, PH, PW], F32, kind="ExternalInput")
    lhsE_d = nc.dram_tensor("lhsE", [K, 128], F32, kind="ExternalInput")
    lhsO_d = nc.dram_tensor("lhsO", [K, 128], F32, kind="ExternalInput")
    bias_d = nc.dram_tensor("bias", [128, 1], F32, kind="ExternalInput")
    out = nc.dram_tensor("out", [PB, OC, HO, WO], F32, kind="ExternalOutput")

    with tile.TileContext(nc) as tc:
        with (
            tc.tile_pool(name="const", bufs=1) as constp,
            tc.tile_pool(name="s", bufs=4) as sp,
            tc.tile_pool(name="v", bufs=3) as vp,
            tc.tile_pool(name="h", bufs=3) as hp,
            tc.tile_pool(name="f", bufs=3) as fp,
            tc.tile_pool(name="ps", bufs=2, space="PSUM") as pp,
        ):
            lE = constp.tile([K, 128], F32, tag="lE")
            nc.sync.dma_start(lE[:], lhsE_d.ap())
            lO = constp.tile([K, 128], F32, tag="lO")
            nc.sync.dma_start(lO[:], lhsO_d.ap())
            bt = constp.tile([128, 1], F32, tag="bias")
            nc.sync.dma_start(bt[:], bias_d.ap())

            for pi in range(PB // 2):       # image pairs
                for tb in range(NT // 2):   # y slab pairs (batched epilogue)
                    Ss = []
                    for s in range(2):
                        y0 = YB * (2 * tb + s)
                        S = sp.tile([K, 448], F32, tag="S")
                        for i in range(2):
                            src = bass.AP(
                                xpad,
                                (2 * pi + i) * PH * PW + y0 * PW,
                                [[1, 5], [PW, KROWS], [1, 224]],
                            )
                            nc.sync.dma_start(S[:, i * 224:(i + 1) * 224], src)
                        Ss.append(S)

                    # 4 PSUM banks: [E0 | O0 | E1 | O1] at 512-elem offsets
                    PS = pp.tile([128, 2048], F32, tag="ps")
                    for s in range(2):
                        nc.tensor.matmul(PS[:, 1024 * s:1024 * s + 448],
                                         lE[:], Ss[s][:], start=True, stop=True)
                        nc.tensor.matmul(PS[:, 1024 * s + 512:1024 * s + 960],
                                         lO[:], Ss[s][:], start=True, stop=True)

                    ps4 = PS[:].rearrange("p (s rest) -> p s rest", s=2)
                    e_view = ps4[:, :, 0:448].rearrange(
                        "p s (i x) -> p s i x", i=2)
                    o_view = ps4[:, :, 512:960].rearrange(
                        "p s (i x) -> p s i x", i=2)

                    # ACT drains the two odd banks to SBUF (DVE cannot read
                    # two PSUM streams in one tensor_tensor)
                    CO = vp.tile([128, 896], F32, tag="CO")
                    co4 = CO[:].rearrange("p (s i x) -> p s i x", s=2, i=2)
                    nc.scalar.copy(co4, o_view)
                    # vertical max: PSUM + SBUF operands
                    V = vp.tile([128, 896], F32, tag="V")
                    v4 = V[:].rearrange("p (s i x) -> p s i x", s=2, i=2)
                    nc.vector.tensor_max(v4, e_view, co4)
                    # horizontal max: strided SBUF
                    Hm = hp.tile([128, 448], F32, tag="H")
                    v5 = V[:].rearrange("p (s i xp two) -> p s i xp two",
                                        s=2, i=2, two=2)
                    h4 = Hm[:].rearrange("p (s i xp) -> p s i xp", s=2, i=2)
                    nc.vector.tensor_max(h4, v5[:, :, :, :, 0],
                                         v5[:, :, :, :, 1])

                    Fo = fp.tile([128, 448], F32, tag="F")
                    nc.scalar.activation(
                        Fo[:], Hm[:], mybir.ActivationFunctionType.Relu,
                        bias=bt[:, 0:1], scale=1.0,
                    )

                    for s in range(2):
                        for i in range(2):
                            dst = bass.AP(
                                out,
                                (2 * pi + i) * OC * HO * WO
                                + (8 * (2 * tb + s)) * WO,
                                [[HO * WO, OC], [WO, 8], [1, WO]],
                            )
                            o0 = s * 224 + i * WO
                            nc.scalar.dma_start(dst, Fo[:, o0:o0 + WO])

    nc.compile()
    return nc


def _host_prep(x, conv_w, conv_b, gamma, beta, run_mean, run_var):
    scale = (gamma / np.sqrt(run_var + BN_EPS)).astype(np.float32)
    wf = (conv_w[:, 0] * scale[:, None, None]).astype(np.float32)       # [16,5,5]
    bf = (conv_b * scale + beta - run_mean * scale).astype(np.float32)  # [16]

    lhsE = np.zeros((K, 128), np.float32)
    lhsO = np.zeros((K, 128), np.float32)
    bias = np.zeros((128, 1), np.float32)
    for o in range(OC):
        for yp in range(8):
            m = o * 8 + yp
            bias[m, 0] = bf[o]
            for j in range(5):
                for dy in range(5):
                    lhsE[j * KROWS + 2 * yp + dy, m] = wf[o, dy, j]
                    lhsO[j * KROWS + 2 * yp + 1 + dy, m] = wf[o, dy, j]

    xpad = np.zeros((B, PH, PW), np.float32)
    xpad[:, 2:2 + H, 2:2 + W] = np.asarray(x, np.float32).reshape(B, H, W)
    return xpad, lhsE, lhsO, bias


def kernel(x, conv_w, conv_b, gamma, beta, run_mean, run_var, _trace=False):
    xpad, lhsE, lhsO, bias = _host_prep(
        x, conv_w, conv_b, gamma, beta, run_mean, run_var
    )
    if "nc" not in _CACHE:
        _CACHE["nc"] = _build_nc()
    nc = _CACHE["nc"]
    in_maps = [
        {
            "xpad": np.ascontiguousarray(xpad[c * PB:(c + 1) * PB]),
            "lhsE": lhsE,
            "lhsO": lhsO,
            "bias": bias,
        }
        for c in range(N_CORES)
    ]
    res = run_bass_kernel_spmd(nc, in_maps, core_ids=list(range(N_CORES)),
                               trace=_trace)
    out = np.concatenate([res.results[c]["out"] for c in range(N_CORES)], axis=0)
    _CACHE["last_results"] = res
    return out
